# revision 1
# baseline (speedup 1.0000x reference)
"""Batchelor GPU-NUFFT forward operator on 8 Trainium2 NeuronCores.

Math (per timepoint t):
    warped  = bilinear_warp(image, flow[..., t])
    coil    = csm * warped                                  [Nc,Nx,Ny]
    out_t[c,s] = sum_{x,y} coil[c,x,y] exp(-2pi i (kx_s (x-64) + ky_s (y-64)))
    out     = sum_t out_t                                   [Nc,NS] complex64

Sharding: 8 cores = 4 timepoints x 2 sample-halves (4096 samples each).
Host unshard: sum the 4 timepoint partials per half, concat halves.

Per-core device algorithm:
  * warp: build interleaved corner table img8[x,y,8] in DRAM (clipping baked
    in), indirect-DMA gather 1 descriptor/pixel, DVE bilinear combine.
  * NUFFT: Khatri-Rao split y = yo*8 + yi. The yi factor is folded into the
    PE contraction: G^T[(c,yo), s] = sum_{x,yi} coil[c,x,yo*8+yi] *
    exp(-2pi i(kx(x-64) + ky(yi-64))), as 32 accumulating bf16 matmuls per
    512-sample chunk (stationary = coil chunks, moving = cos/sin tiles from
    the ACT Sin spline; range reduction via the +-2^23 round trick since the
    Sin table only covers [-pi, pi] and the DVE has no mod).
  * Outer factor A[(c,yo), s] = exp(-2pi i ky 8 yo) applied elementwise on
    DVE, then the yo-reduction (a partition-dim reduction) is done on the PE
    with a constant 0/1 selector matmul. Output [8, 4096] f32 pair per core.
"""

import sys

if "/opt/trn_rl_repo" not in sys.path:
    sys.path.insert(0, "/opt/trn_rl_repo")

import math

import numpy as np

import concourse.bass as bass
import concourse.tile as tile
from concourse import bacc
from concourse import mybir

P = 128
NX = 128
NCOIL = 8
NS = 8192
NT = 4
S = 4096  # samples per core (half of NS)
CH = 512  # samples per inner chunk
NCHUNK = S // CH
YI = 8
YO = 16

F32 = mybir.dt.float32
BF16 = mybir.dt.bfloat16
F32R = mybir.dt.float32r
I32 = mybir.dt.int32
TWO_PI = float(2.0 * math.pi)
MAGIC = 12582912.0  # 1.5*2^23: (x + M) - M == round-to-nearest(x), sum stays in [2^23, 2^24)
ALU = mybir.AluOpType
ACTF = mybir.ActivationFunctionType


def build_program(nc: bass.Bass, dbg: bool = False):
    def dbg_out(name, src_ap, shape, dtype=F32):
        if not dbg:
            return
        d = nc.dram_tensor("dbg_" + name, shape, dtype, kind="ExternalOutput").ap()
        nc.sync.dma_start(d[:], src_ap)

    image_r = nc.dram_tensor("image_r", [NX, NX], F32, kind="ExternalInput").ap()
    image_i = nc.dram_tensor("image_i", [NX, NX], F32, kind="ExternalInput").ap()
    csm_r = nc.dram_tensor("csm_r", [NCOIL, NX, NX], F32, kind="ExternalInput").ap()
    csm_i = nc.dram_tensor("csm_i", [NCOIL, NX, NX], F32, kind="ExternalInput").ap()
    kx_d = nc.dram_tensor("kx", [S], F32, kind="ExternalInput").ap()
    ky_d = nc.dram_tensor("ky", [S], F32, kind="ExternalInput").ap()
    flow0_d = nc.dram_tensor("flow0", [NX, NX], F32, kind="ExternalInput").ap()
    flow1_d = nc.dram_tensor("flow1", [NX, NX], F32, kind="ExternalInput").ap()
    out_r = nc.dram_tensor("out_r", [NCOIL, S], F32, kind="ExternalOutput").ap()
    out_i = nc.dram_tensor("out_i", [NCOIL, S], F32, kind="ExternalOutput").ap()
    img8_d = nc.dram_tensor("img8_scratch", [NX * NX, 8], F32, kind="Internal").ap()

    # ---------------- inline constants ----------------
    pvals = np.arange(P, dtype=np.float32)
    iota_pf_d = nc.inline_tensor(pvals.reshape(P, 1), name="c_iota_pf").ap()
    xc_d = nc.inline_tensor((pvals - 64.0).reshape(P, 1), name="c_xc").ap()
    yo8_d = nc.inline_tensor((8.0 * (np.arange(P) % 16)).astype(np.float32)
                             .reshape(P, 1), name="c_yo8").ap()
    half_pi_d = nc.inline_tensor(np.full((P, 1), math.pi / 2, np.float32),
                                 name="c_half_pi").ap()
    jrow_d = nc.inline_tensor(np.tile(np.arange(NX, dtype=np.float32), (P, 1)),
                              name="c_jrow").ap()
    sel_np = (np.arange(P)[:, None] // 16 == np.arange(NCOIL)[None, :]).astype(
        np.float32)
    sel_d = nc.inline_tensor(sel_np, name="c_sel").ap()

    with tile.TileContext(nc) as tc, \
         tc.tile_pool(name="pp", bufs=1) as pp, \
         tc.tile_pool(name="big", bufs=1) as bp:

        iota_pf = pp.tile([P, 1], F32)
        nc.sync.dma_start(iota_pf[:], iota_pf_d[:])
        xc_col = pp.tile([P, 1], F32)
        nc.sync.dma_start(xc_col[:], xc_d[:])
        yo8 = pp.tile([P, 1], F32)
        nc.sync.dma_start(yo8[:], yo8_d[:])
        half_pi = pp.tile([P, 1], F32)
        nc.sync.dma_start(half_pi[:], half_pi_d[:])
        jrow = pp.tile([P, NX], F32)
        nc.sync.dma_start(jrow[:], jrow_d[:])
        self32 = pp.tile([P, NCOIL], F32)
        nc.sync.dma_start(self32[:], sel_d[:])
        sel = pp.tile([P, NCOIL], F32R)
        nc.vector.tensor_copy(sel[:], self32[:])

        # persistent big tiles
        RA = bp.tile([P, YI, 256], F32R)
        RB = bp.tile([P, YI, 256], F32R)
        if dbg:
            pass
        kxb = bp.tile([P, S], F32)
        nc.sync.dma_start(
            kxb[:], kx_d.rearrange("(p s) -> p s", p=1).to_broadcast([P, S]))
        kyb = bp.tile([P, S], F32)
        nc.sync.dma_start(
            kyb[:], ky_d.rearrange("(p s) -> p s", p=1).to_broadcast([P, S]))

        # ================ warp + coil (scoped pool) ================
        with tc.tile_pool(name="warp", bufs=1) as wp:
            fl0 = wp.tile([P, NX], F32)
            nc.sync.dma_start(fl0[:], flow0_d[:])
            fl1 = wp.tile([P, NX], F32)
            nc.sync.dma_start(fl1[:], flow1_d[:])
            img_r_sb = wp.tile([P, NX], F32)
            nc.sync.dma_start(img_r_sb[:], image_r[:])
            img_i_sb = wp.tile([P, NX], F32)
            nc.sync.dma_start(img_i_sb[:], image_i[:])

            cx = wp.tile([P, NX], F32)
            nc.vector.tensor_scalar(cx[:], fl0[:], iota_pf[:, 0:1], None, op0=ALU.add)
            cx2 = wp.tile([P, NX], F32)
            nc.vector.tensor_scalar(cx2[:], cx[:], 127.0, 0.0, op0=ALU.min, op1=ALU.max)
            cyt = wp.tile([P, NX], F32)
            nc.vector.tensor_tensor(cyt[:], fl1[:], jrow[:], op=ALU.add)
            cy2 = wp.tile([P, NX], F32)
            nc.vector.tensor_scalar(cy2[:], cyt[:], 127.0, 0.0, op0=ALU.min, op1=ALU.max)

            # floor via round(x - 0.5); ties resolve to either neighbour with
            # weight 0/1 which gives an identical bilinear result
            c5x = wp.tile([P, NX], F32)
            nc.vector.tensor_scalar(c5x[:], cx2[:], 0.5, None, op0=ALU.subtract)
            x0 = wp.tile([P, NX], F32)
            nc.vector.tensor_scalar(x0[:], c5x[:], MAGIC, MAGIC,
                                    op0=ALU.add, op1=ALU.subtract)
            wx = wp.tile([P, NX], F32)
            nc.vector.tensor_tensor(wx[:], cx2[:], x0[:], op=ALU.subtract)
            c5y = wp.tile([P, NX], F32)
            nc.vector.tensor_scalar(c5y[:], cy2[:], 0.5, None, op0=ALU.subtract)
            y0 = wp.tile([P, NX], F32)
            nc.vector.tensor_scalar(y0[:], c5y[:], MAGIC, MAGIC,
                                    op0=ALU.add, op1=ALU.subtract)
            wy = wp.tile([P, NX], F32)
            nc.vector.tensor_tensor(wy[:], cy2[:], y0[:], op=ALU.subtract)

            idxf = wp.tile([P, NX], F32)
            nc.vector.tensor_scalar(idxf[:], x0[:], 128.0, None, op0=ALU.mult)
            idxf2 = wp.tile([P, NX], F32)
            nc.vector.tensor_tensor(idxf2[:], idxf[:], y0[:], op=ALU.add)
            idx_i = wp.tile([P, NX], I32)
            nc.vector.tensor_copy(idx_i[:], idxf2[:])

            # x+1 row-shifted copies (clipped at row 127)
            imgBr = wp.tile([P, NX], F32)
            nc.sync.dma_start(imgBr[0:127, :], img_r_sb[1:128, :])
            nc.sync.dma_start(imgBr[127:128, :], img_r_sb[127:128, :])
            imgBi = wp.tile([P, NX], F32)
            nc.sync.dma_start(imgBi[0:127, :], img_i_sb[1:128, :])
            nc.sync.dma_start(imgBi[127:128, :], img_i_sb[127:128, :])

            # interleaved corner table [x, y, 8]
            img8 = wp.tile([P, NX, 8], F32)
            for k, src in ((0, img_r_sb), (2, imgBr), (4, img_i_sb), (6, imgBi)):
                nc.vector.tensor_copy(img8[:, :, k], src[:])
                nc.vector.tensor_copy(img8[:, 0:127, k + 1], src[:, 1:128])
                nc.vector.tensor_copy(img8[:, 127:128, k + 1], src[:, 127:128])
            nc.sync.dma_start(
                img8_d.rearrange("(x y) k -> x (y k)", x=NX), img8[:])

            # per-pixel gather: the [P,1]-index form is the only one the HW
            # SWDGE handles correctly (production scatter_add form), so loop
            # over pixel columns, one indirect DMA per column.
            g8 = wp.tile([P, NX, 8], F32)
            for j in range(NX):
                nc.gpsimd.indirect_dma_start(
                    out=g8[:, j, :],
                    out_offset=None,
                    in_=img8_d[:],
                    in_offset=bass.IndirectOffsetOnAxis(ap=idx_i[:, j:j + 1], axis=0),
                )

            onemwx = wp.tile([P, NX], F32)
            nc.vector.tensor_scalar(onemwx[:], wx[:], -1.0, 1.0, op0=ALU.mult, op1=ALU.add)
            onemwy = wp.tile([P, NX], F32)
            nc.vector.tensor_scalar(onemwy[:], wy[:], -1.0, 1.0, op0=ALU.mult, op1=ALU.add)
            w4 = wp.tile([P, NX, 4], F32)
            nc.vector.tensor_tensor(w4[:, :, 0], onemwx[:], onemwy[:], op=ALU.mult)
            nc.vector.tensor_tensor(w4[:, :, 1], onemwx[:], wy[:], op=ALU.mult)
            nc.vector.tensor_tensor(w4[:, :, 2], wx[:], onemwy[:], op=ALU.mult)
            nc.vector.tensor_tensor(w4[:, :, 3], wx[:], wy[:], op=ALU.mult)

            t8r = wp.tile([P, NX, 4], F32)
            nc.vector.tensor_tensor(t8r[:], g8[:, :, 0:4], w4[:], op=ALU.mult)
            warped_r = wp.tile([P, NX], F32)
            nc.vector.reduce_sum(warped_r[:], t8r[:], axis=mybir.AxisListType.X)
            t8i = wp.tile([P, NX, 4], F32)
            nc.vector.tensor_tensor(t8i[:], g8[:, :, 4:8], w4[:], op=ALU.mult)
            warped_i = wp.tile([P, NX], F32)
            nc.vector.reduce_sum(warped_i[:], t8i[:], axis=mybir.AxisListType.X)
            dbg_out("warped_r", warped_r[:], [P, NX])
            dbg_out("warped_i", warped_i[:], [P, NX])
            dbg_out("g8", g8[:], [P, NX, 8])
            dbg_out("idx", idx_i[:], [P, NX], I32)
            dbg_out("x0", x0[:], [P, NX])
            dbg_out("wx", wx[:], [P, NX])

            # ---- coil = csm * warped, packed for the PE ----
            csm_r_sb = wp.tile([P, NCOIL, NX], F32)
            nc.sync.dma_start(csm_r_sb[:], csm_r.rearrange("c x y -> x c y"))
            csm_i_sb = wp.tile([P, NCOIL, NX], F32)
            nc.sync.dma_start(csm_i_sb[:], csm_i.rearrange("c x y -> x c y"))

            wr_b = warped_r[:].rearrange("p (c y) -> p c y", c=1).to_broadcast(
                [P, NCOIL, NX])
            wi_b = warped_i[:].rearrange("p (c y) -> p c y", c=1).to_broadcast(
                [P, NCOIL, NX])

            tt1 = wp.tile([P, NCOIL, NX], F32)
            nc.vector.tensor_tensor(tt1[:], csm_r_sb[:], wr_b, op=ALU.mult)
            tt2 = wp.tile([P, NCOIL, NX], F32)
            nc.vector.tensor_tensor(tt2[:], csm_i_sb[:], wi_b, op=ALU.mult)
            coilr = wp.tile([P, NCOIL, NX], F32)
            nc.vector.tensor_tensor(coilr[:], tt1[:], tt2[:], op=ALU.subtract)
            tt3 = wp.tile([P, NCOIL, NX], F32)
            nc.vector.tensor_tensor(tt3[:], csm_r_sb[:], wi_b, op=ALU.mult)
            tt4 = wp.tile([P, NCOIL, NX], F32)
            nc.vector.tensor_tensor(tt4[:], csm_i_sb[:], wr_b, op=ALU.mult)
            coili = wp.tile([P, NCOIL, NX], F32)
            nc.vector.tensor_tensor(coili[:], tt3[:], tt4[:], op=ALU.add)
            dbg_out("coilr", coilr[:], [P, NCOIL, NX])
            dbg_out("coili", coili[:], [P, NCOIL, NX])

            def coil_view(t):
                return t[:].rearrange("p c (yo yi) -> p c yo yi", yi=YI)

            def pack_view(t, r):
                return t[:].rearrange("p yi (r c yo) -> p r c yo yi",
                                      r=2, c=NCOIL)[:, r]

            nc.vector.tensor_copy(pack_view(RA, 0), coil_view(coilr))
            nc.vector.tensor_copy(pack_view(RA, 1), coil_view(coili))
            nc.vector.tensor_scalar(pack_view(RB, 0), coil_view(coili), -1.0, None,
                                    op0=ALU.mult)
            nc.vector.tensor_copy(pack_view(RB, 1), coil_view(coilr))

        # ================ main chunk loop ================
        with tc.tile_pool(name="loop", bufs=1) as lp, \
             tc.tile_pool(name="kr", bufs=1) as kp, \
             tc.tile_pool(name="ps", bufs=2, space="PSUM") as ps, \
             tc.tile_pool(name="pso", bufs=1, space="PSUM") as pso:

            def frac_centered(src_ap, rtag, mtag, bufs=1):
                """m2 = src - round(src) in [-0.5, 0.5]."""
                r = lp.tile([P, CH], F32, tag=rtag, bufs=bufs)
                nc.vector.tensor_scalar(r[:], src_ap, MAGIC, MAGIC,
                                        op0=ALU.add, op1=ALU.subtract)
                m2 = lp.tile([P, CH], F32, tag=mtag, bufs=bufs)
                nc.vector.tensor_tensor(m2[:], src_ap, r[:], op=ALU.subtract)
                return m2

            for ch in range(NCHUNK):
                c0 = ch * CH
                kxc = kxb[:, c0:c0 + CH]
                kyc = kyb[:, c0:c0 + CH]

                # outer factor A^T[(c,yo), s] for this chunk
                ta = lp.tile([P, CH], F32, tag="ta")
                nc.vector.tensor_scalar(ta[:], kyc, yo8[:, 0:1], None, op0=ALU.mult)
                m2a = frac_centered(ta[:], "ra", "m2a")
                mabsa = lp.tile([P, CH], F32, tag="mabsa")
                nc.scalar.activation(mabsa[:], m2a[:], ACTF.Abs)
                aic = lp.tile([P, CH], F32, tag="aic")
                nc.scalar.activation(aic[:], m2a[:], ACTF.Sin, scale=-TWO_PI)
                arc = lp.tile([P, CH], F32, tag="arc")
                nc.scalar.activation(arc[:], mabsa[:], ACTF.Sin, scale=-TWO_PI,
                                     bias=half_pi[:, 0:1])

                # moving-operand cos/sin tiles (Khatri-Rao inner factor)
                u = lp.tile([P, CH], F32, tag="u")
                nc.vector.tensor_scalar(u[:], kxc, xc_col[:, 0:1], None, op0=ALU.mult)
                b64 = lp.tile([P, CH], F32, tag="b64")
                nc.vector.tensor_scalar(b64[:], kyc, -64.0, None, op0=ALU.mult)

                krr = []
                kri = []
                prev = None
                for yi in range(YI):
                    argt = lp.tile([P, CH], F32, tag="arg", bufs=3)
                    if yi == 0:
                        nc.vector.tensor_tensor(argt[:], u[:], b64[:], op=ALU.add)
                    else:
                        nc.vector.tensor_tensor(argt[:], prev[:], kyc, op=ALU.add)
                    m2 = frac_centered(argt[:], "rk", "m2k", bufs=2)
                    mabs = lp.tile([P, CH], F32, tag="mabsk", bufs=2)
                    nc.scalar.activation(mabs[:], m2[:], ACTF.Abs)
                    kit = kp.tile([P, CH], F32R, tag=f"kri{yi}")
                    nc.scalar.activation(kit[:], m2[:], ACTF.Sin, scale=-TWO_PI)
                    krt = kp.tile([P, CH], F32R, tag=f"krr{yi}")
                    nc.scalar.activation(krt[:], mabs[:], ACTF.Sin, scale=-TWO_PI,
                                         bias=half_pi[:, 0:1])
                    if ch == 0:
                        dbg_out(f"kri{yi}", kit[:], [P, CH])
                        dbg_out(f"krr{yi}", krt[:], [P, CH])
                        dbg_out(f"m2k{yi}", m2[:], [P, CH])
                    kri.append(kit)
                    krr.append(krt)
                    prev = argt

                gr = ps.tile([P, CH], F32, tag="gr")
                gi = ps.tile([P, CH], F32, tag="gi")
                for yi in range(YI):
                    nc.tensor.matmul(gr[:], RA[:, yi, 0:128], krr[yi][:],
                                     start=(yi == 0), stop=False)
                    nc.tensor.matmul(gr[:], RB[:, yi, 0:128], kri[yi][:],
                                     start=False, stop=(yi == YI - 1))
                for yi in range(YI):
                    nc.tensor.matmul(gi[:], RA[:, yi, 128:256], krr[yi][:],
                                     start=(yi == 0), stop=False)
                    nc.tensor.matmul(gi[:], RB[:, yi, 128:256], kri[yi][:],
                                     start=False, stop=(yi == YI - 1))

                if ch == 0:
                    grs = lp.tile([P, CH], F32, tag="dbg_gr")
                    nc.vector.tensor_copy(grs[:], gr[:])
                    dbg_out("gr0", grs[:], [P, CH])
                    gis = lp.tile([P, CH], F32, tag="dbg_gi")
                    nc.vector.tensor_copy(gis[:], gi[:])
                    dbg_out("gi0", gis[:], [P, CH])
                    dbg_out("arc0", arc[:], [P, CH])
                    dbg_out("aic0", aic[:], [P, CH])
                t1 = lp.tile([P, CH], F32, tag="s2a")
                nc.vector.tensor_tensor(t1[:], gr[:], arc[:], op=ALU.mult)
                t2 = lp.tile([P, CH], F32, tag="s2b")
                nc.vector.tensor_tensor(t2[:], gi[:], aic[:], op=ALU.mult)
                pr = lp.tile([P, CH], F32R, tag="pr")
                nc.vector.tensor_tensor(pr[:], t1[:], t2[:], op=ALU.subtract)
                t3 = lp.tile([P, CH], F32, tag="s2c")
                nc.vector.tensor_tensor(t3[:], gi[:], arc[:], op=ALU.mult)
                t4 = lp.tile([P, CH], F32, tag="s2d")
                nc.vector.tensor_tensor(t4[:], gr[:], aic[:], op=ALU.mult)
                pi_ = lp.tile([P, CH], F32R, tag="pi")
                nc.vector.tensor_tensor(pi_[:], t3[:], t4[:], op=ALU.add)

                orps = pso.tile([NCOIL, CH], F32, tag="or")
                nc.tensor.matmul(orps[:], sel[:], pr[:], start=True, stop=True)
                oips = pso.tile([NCOIL, CH], F32, tag="oi")
                nc.tensor.matmul(oips[:], sel[:], pi_[:], start=True, stop=True)

                osr = lp.tile([NCOIL, CH], F32, tag="osr", bufs=2)
                nc.scalar.copy(osr[:], orps[:])
                osi = lp.tile([NCOIL, CH], F32, tag="osi", bufs=2)
                nc.scalar.copy(osi[:], oips[:])
                nc.sync.dma_start(out_r[:, c0:c0 + CH], osr[:])
                nc.sync.dma_start(out_i[:, c0:c0 + CH], osi[:])


_COMPILED = {}


def _get_nc():
    if "nc" not in _COMPILED:
        nc = bacc.Bacc("TRN2", debug=False)
        build_program(nc)
        nc.compile()
        _COMPILED["nc"] = nc
    return _COMPILED["nc"]


def make_in_maps(image_r, image_i, csm_r, csm_i, traj, dcf, flow):
    del dcf  # unused by the operator
    in_maps = []
    for core in range(8):
        t, h = divmod(core, 2)
        sl = slice(h * S, (h + 1) * S)
        in_maps.append({
            "image_r": np.ascontiguousarray(image_r, np.float32),
            "image_i": np.ascontiguousarray(image_i, np.float32),
            "csm_r": np.ascontiguousarray(csm_r, np.float32),
            "csm_i": np.ascontiguousarray(csm_i, np.float32),
            "kx": np.ascontiguousarray(traj[sl, 0, t], np.float32),
            "ky": np.ascontiguousarray(traj[sl, 1, t], np.float32),
            "flow0": np.ascontiguousarray(flow[:, :, 0, t], np.float32),
            "flow1": np.ascontiguousarray(flow[:, :, 1, t], np.float32),
        })
    return in_maps


def combine_outputs(results):
    out = np.zeros((NCOIL, NS), np.complex64)
    for core, res in enumerate(results):
        t, h = divmod(core, 2)
        sl = slice(h * S, (h + 1) * S)
        out[:, sl] += res["out_r"].astype(np.complex64) + 1j * res["out_i"].astype(
            np.complex64)
    return out


def kernel(**inputs) -> np.ndarray:
    from concourse.bass_utils import run_bass_kernel_spmd

    nc = _get_nc()
    in_maps = make_in_maps(**inputs)
    res = run_bass_kernel_spmd(nc, in_maps, core_ids=list(range(8)))
    return combine_outputs(res.results)

